# revision 17
# baseline (speedup 1.0000x reference)
"""DFSMN layer Trainium2 kernel (8-core SPMD, batch-parallel).

Math: per batch b,
  h = x @ W^T + b_lin                      [L, H]
  out_pre[t] = h[t] + mem[t] + fut[t]  ==  (M @ h)[t]
    with M [L, L] banded: identity + past taps (50) + future taps (5),
    taps are scalars per lag: wm = mem_w.sum(-1), wf = la_w.sum(-1).
  out = LayerNorm_H(out_pre) * gamma + beta

On device (per core = one batch):
  Stage A produces g on a grid SHIFTED by -56: E_j holds g rows
  t in [128j-56, 128j+72); the edge halves (t in [1992,2048) and
  [0,72)) pack exactly into ONE merged tile gm (=tile 0, 56+72=128
  partitions, no padding). The Linear bias is folded into the stage-A
  PSUM evacuation as a DVE broadcast add (b shipped pre-tiled to
  [128, H]).

  Band: output tile j's UPPER 64 rows (t in [128j,128j+64)) need src
  t in [128j-50, 128j+68] which E_j covers; the LOWER 64 rows need
  src in [128j+14, 128j+132] covered by O_j = [128j+8, 128j+136),
  assembled from E_j[64:128] and E_{j+1}[0:64] by two partition-
  shifting SBUF->SBUF DMAs. So the band is TWO M=64 K=128 matmuls per
  (tile, h-chunk) issued to different PE column groups (out strips
  0:63 / 64:127) which stream CONCURRENTLY -- about one matmul-span
  of PE time instead of three matmuls + a rank-1 bias matmul.
  LayerNorm via DVE bn_stats/bn_aggr as before.
"""
import numpy as np
import ml_dtypes

MEM, LA, EPS = 50, 5, 1e-5
B, L, D, H = 8, 2048, 1024, 2048
NCORES = 8
PT = 128              # time tile (partition dim)
TB = L // PT          # 16 output time tiles
DC = D // PT          # 8 contract chunks
HN = 512              # matmul moving free dim
HC = H // HN          # 4 H chunks
OFF = 56              # source grid shift: E_j = [128j-OFF, 128j+128-OFF)
HEAD = PT - OFF       # 72: head rows of tile 0; tail rows = OFF = 56
NMT = 2 * TB          # band blocks: (ma_j, mb_j) per output tile

_cached = {}
last_exec_time_ns = None


def _band_matrix(wm, wf):
    """M [L, L] fp32: out_pre = M @ h."""
    M = np.zeros((L, L), np.float32)
    idx = np.arange(L)
    M[idx, idx] = 1.0
    for t in range(L):
        if t < MEM:
            M[t, :t] += wm[:t]
        else:
            M[t, t - MEM:t] += wm
        hi = min(t + LA, L - 1)
        if hi >= t + 1:
            M[t, t + 1:hi + 1] += wf[:hi - t]
    return M


def _src_t(k, p):
    """t index held at partition p of source tile k (merged gm at k=0:
    partitions 0..OFF-1 hold the tail t in [L-OFF, L), partitions
    OFF..127 hold the head t in [0, HEAD)). Returns None if out of
    range [0, L)."""
    if k == 0:
        t = (L - OFF + p) if p < OFF else (p - OFF)
    else:
        t = 128 * k - OFF + p
    return t if 0 <= t < L else None


def _build_nc(reps=1, loop_k=None):
    from concourse import bacc
    import concourse.mybir as mybir
    import concourse.tile as tile

    dt = mybir.dt.bfloat16
    f32 = mybir.dt.float32
    sub = mybir.AluOpType.subtract
    mult = mybir.AluOpType.mult
    add = mybir.AluOpType.add

    nc = bacc.Bacc(None, target_bir_lowering=False)
    # x shipped pre-transposed, shifted-tile-major, partition(d%128)-major:
    # xsT[k] is [128, DC*PT] with per-partition-contiguous 2KB lines.
    xsT = nc.declare_dram_parameter("xsT", [TB, PT, DC * PT], dt, isOutput=False)
    wT = nc.declare_dram_parameter("wT", [D, H], dt, isOutput=False)
    mT = nc.declare_dram_parameter("mT", [PT, NMT, 64], dt, isOutput=False)
    bf = nc.declare_dram_parameter("bf", [PT, H], dt, isOutput=False)
    out = nc.declare_dram_parameter("out", [L, H], f32, isOutput=True)

    with tile.TileContext(nc) as tc:
        with tc.tile_pool(name="const", bufs=1) as const, \
             tc.tile_pool(name="gpool", bufs=4) as gpool, \
             tc.tile_pool(name="owin", bufs=3) as owin, \
             tc.tile_pool(name="opool", bufs=3) as opool, \
             tc.tile_pool(name="ln", bufs=2) as ln, \
             tc.tile_pool(name="psg", bufs=4, space="PSUM") as psg, \
             tc.tile_pool(name="psp", bufs=3, space="PSUM") as psp:

            # Input DMAs round-robin over 4 engine queues, first-needed
            # first: tile 0's x slice + the hc=0 weight chunks gate the
            # first matmul chain (~1.25MB), everything else streams in
            # behind compute.
            qs = [nc.sync, nc.scalar, nc.gpsimd]
            qi = 0

            def q():
                nonlocal qi
                e = qs[qi % len(qs)]
                qi += 1
                return e

            xs_tiles = []
            for k in range(TB):
                t = const.tile([PT, DC, PT], dt, tag=f"xs{k}")
                xs_tiles.append(t)
            wt_tiles = {}
            for hc in range(HC):
                for dc in range(DC):
                    w = const.tile([PT, HN], dt, tag=f"wt{dc}_{hc}")
                    wt_tiles[(dc, hc)] = w

            q().dma_start(out=xs_tiles[0],
                          in_=xsT[0].rearrange("p (dc t) -> p dc t", dc=DC))
            for dc in range(DC):
                q().dma_start(out=wt_tiles[(dc, 0)],
                              in_=wT[dc * PT:(dc + 1) * PT, 0:HN])
            q().dma_start(out=xs_tiles[1],
                          in_=xsT[1].rearrange("p (dc t) -> p dc t", dc=DC))
            for hc in range(1, HC):
                for dc in range(DC):
                    q().dma_start(out=wt_tiles[(dc, hc)],
                                  in_=wT[dc * PT:(dc + 1) * PT,
                                         hc * HN:(hc + 1) * HN])
            for k in range(2, TB):
                q().dma_start(out=xs_tiles[k],
                              in_=xsT[k].rearrange("p (dc t) -> p dc t", dc=DC))
            mt_t = const.tile([PT, NMT, 64], dt, tag="mt")
            q().dma_start(out=mt_t, in_=mT[:, :, :])
            bf_t = const.tile([PT, H], dt, tag="bf")
            q().dma_start(out=bf_t, in_=bf[:, :])
            eps_t = const.tile([PT, 1], f32, tag="eps")
            nc.vector.memset(eps_t, EPS)

            consts = (xs_tiles, wt_tiles, mt_t, bf_t, eps_t)
            pools = (gpool, owin, opool, ln, psg, psp)
            ops = (sub, mult, add)
            if loop_k is not None:
                # For_i places an all-engine barrier at each iteration
                # boundary (PE sits idle through the LN/DMA tail, then
                # restarts cold). Unroll so the barrier is paid once per
                # UNROLL bodies; in between, bodies pipeline through the
                # tile rings.
                UNROLL = 8
                assert loop_k % UNROLL == 0
                with tc.For_i(0, loop_k // UNROLL, 1):
                    for _u in range(UNROLL):
                        _emit_body(nc, mybir, consts, pools, out, ops)
            else:
                for _rep in range(reps):
                    _emit_body(nc, mybir, consts, pools, out, ops)
    nc.finalize()
    return nc


def _emit_body(nc, mybir, consts, pools, out, ops):
    dt = mybir.dt.bfloat16
    f32 = mybir.dt.float32
    sub, mult, add = ops
    xs_tiles, wt_tiles, mt_t, bf_t, eps_t = consts
    gpool, owin, opool, ln, psg, psp = pools
    oq = [nc.sync, nc.scalar, nc.gpsimd]

    # g source tiles: k=0 (gm) lives in its own buffers for the whole
    # body (read by band j=0 AND the O_15 window); k=1..15 rotate.
    g_sb = [None] * TB

    def emit_A(k):
        gch = []
        for hc in range(HC):
            pg = psg.tile([PT, HN], f32, tag="pg")
            for dc in range(DC):
                nc.tensor.matmul(
                    pg,
                    xs_tiles[k][:, dc, :],
                    wt_tiles[(dc, hc)],
                    start=(dc == 0), stop=(dc == DC - 1))
            # gm (k=0) gets its own tags: it must survive until band j=15.
            tag = f"gm{hc}" if k == 0 else f"g{hc}"
            g = gpool.tile([PT, HN], dt, tag=tag)
            # Fold the Linear bias into the evacuation: g = psum + b.
            nc.vector.tensor_tensor(
                out=g, in0=pg, in1=bf_t[:, hc * HN:(hc + 1) * HN], op=add)
            gch.append(g)
        g_sb[k] = gch

    def emit_B(j):
        # O_j window: partitions 0:64 <- E_j[64:128], 64:128 <- E_{j+1}[0:64]
        # (j=15: 64:120 <- gm[0:56]; the K=120 matmul never reads 120:128).
        kb = 64 + OFF if j == TB - 1 else PT   # valid lower-src partitions
        o_ch = []
        for hc in range(HC):
            ow = owin.tile([PT, HN], dt, tag=f"ow{hc}")
            oq[(j + hc) % 3].dma_start(out=ow[0:64, :],
                                       in_=g_sb[j][hc][64:128, :])
            src = g_sb[0][hc][0:OFF, :] if j == TB - 1 \
                else g_sb[j + 1][hc][0:64, :]
            oq[(j + hc + 1) % 3].dma_start(out=ow[64:kb, :], in_=src)
            o_ch.append(ow)

        stats = ln.tile([PT, HC, 6], f32, tag="stats")
        presb_ch = []
        for hc in range(HC):
            pre = psp.tile([PT, HN], f32, tag="pre")
            # Upper/lower 64-row halves on different PE column groups --
            # they stream concurrently (out strips 0:63 and 64:127).
            nc.tensor.matmul(pre[0:64, :], mt_t[:, 2 * j, :], g_sb[j][hc],
                             start=True, stop=True)
            nc.tensor.matmul(pre[64:128, :], mt_t[0:kb, 2 * j + 1, :],
                             o_ch[hc][0:kb, :], start=True, stop=True,
                             skip_group_check=True)
            # Evacuate PSUM on ScalarE (close to PSUM); LN from SBUF.
            pre_sb = opool.tile([PT, HN], f32, tag=f"presb{hc}")
            nc.scalar.copy(out=pre_sb, in_=pre)
            nc.vector.bn_stats(out=stats[:, hc, :], in_=pre_sb)
            presb_ch.append(pre_sb)
        mv = ln.tile([PT, 2], f32, tag="mv")
        nc.vector.bn_aggr(out=mv, in_=stats)
        rstd = ln.tile([PT, 1], f32, tag="rstd")
        nc.scalar.activation(
            out=rstd, in_=mv[:, 1:2],
            func=mybir.ActivationFunctionType.Sqrt,
            bias=eps_t, scale=1.0)
        nc.vector.reciprocal(out=rstd, in_=rstd)
        # Per-chunk LN apply + chunked out-DMA on rotating queues: the
        # DMA of chunk hc overlaps the tensor_scalar of chunk hc+1, and
        # the iteration tail is one 256KB DMA instead of a 1MB one.
        o = opool.tile([PT, HC, HN], f32, tag="o")
        for hc in range(HC):
            nc.vector.tensor_scalar(
                out=o[:, hc, :], in0=presb_ch[hc],
                scalar1=mv[:, 0:1], scalar2=rstd,
                op0=sub, op1=mult)
            oq[(4 * j + hc) % 3].dma_start(
                out=out[j * PT:(j + 1) * PT, hc * HN:(hc + 1) * HN],
                in_=o[:, hc, :])

    # A(0), A(1), B(0), A(2), B(1), ..., A(15), B(14), B(15)
    emit_A(0)
    for k in range(1, TB):
        emit_A(k)
        emit_B(k - 1)
    emit_B(TB - 1)


def _get_runner(reps=1):
    """Compile once; return (run_fn, in_names, out_names).

    run_fn takes a list of global (concatenated-over-cores) jax/np arrays in
    in_names order followed by zero output buffers, returns global outputs.
    Mirrors concourse.bass2jax.run_bass_via_pjrt's multi-core branch, but
    keeps the jitted callable so repeated invocations don't rebuild/retrace.
    """
    key = ("runner", reps)
    if key in _cached:
        return _cached[key]

    import jax
    from jax.experimental.shard_map import shard_map
    from jax.sharding import Mesh, PartitionSpec
    import concourse.mybir as mybir
    from concourse import bass2jax

    if isinstance(reps, tuple):  # ("loop", K): hardware For_i timing variant
        nc = _build_nc(loop_k=reps[1])
    else:
        nc = _build_nc(reps)
    bass2jax.install_neuronx_cc_hook()

    partition_name = nc.partition_id_tensor.name if nc.partition_id_tensor else None
    in_names, out_names, out_avals, zero_outs = [], [], [], []
    for alloc in nc.m.functions[0].allocations:
        if not isinstance(alloc, mybir.MemoryLocationSet):
            continue
        name = alloc.memorylocations[0].name
        if alloc.kind == "ExternalInput":
            if name != partition_name:
                in_names.append(name)
        elif alloc.kind == "ExternalOutput":
            out_names.append(name)
            shape = tuple(alloc.tensor_shape)
            dtype = mybir.dt.np(alloc.dtype)
            out_avals.append(jax.core.ShapedArray(shape, dtype))
            zero_outs.append(np.zeros(shape, dtype))
    n_params = len(in_names)
    all_names = in_names + out_names
    if partition_name is not None:
        all_names.append(partition_name)

    def _body(*args):
        operands = list(args)
        if partition_name is not None:
            operands.append(bass2jax.partition_id_tensor())
        outs = bass2jax._bass_exec_p.bind(
            *operands,
            out_avals=tuple(out_avals),
            in_names=tuple(all_names),
            out_names=tuple(out_names),
            lowering_input_output_aliases=(),
            sim_require_finite=True,
            sim_require_nnan=True,
            nc=nc,
        )
        return tuple(outs)

    devices = jax.devices()[:NCORES]
    assert len(devices) == NCORES, f"need {NCORES} devices, have {len(jax.devices())}"
    mesh = Mesh(np.asarray(devices), ("core",))
    n_outs = len(out_names)
    fn = jax.jit(shard_map(
        _body, mesh=mesh,
        in_specs=(PartitionSpec("core"),) * (n_params + n_outs),
        out_specs=(PartitionSpec("core"),) * n_outs,
        check_rep=False))

    _cached[key] = (fn, in_names, out_names, zero_outs, mesh)
    return _cached[key]


def _prepare_in_arrays(x, W_lin, b_lin, wm, wf):
    """Host prep: per-core inputs concatenated over the core axis (axis 0)."""
    bf16 = ml_dtypes.bfloat16
    M = _band_matrix(wm, wf)
    # ma_j[p, q] = M[128j+q,       src_t(j, p)]   (upper 64 out rows, E_j)
    # mb_j[p, q] = M[128j+64+q, 128j+8+p]         (lower 64 out rows, O_j)
    mt_host = np.zeros((PT, NMT, 64), np.float32)
    for j in range(TB):
        for p in range(PT):
            t = _src_t(j, p)
            if t is not None:
                mt_host[p, 2 * j, :] = M[j * PT:j * PT + 64, t]
        lo = 128 * j + 8
        n = min(PT, L - lo)                        # 120 for j=15
        mt_host[0:n, 2 * j + 1, :] = M[j * PT + 64:(j + 1) * PT, lo:lo + n].T
    per_core = {
        "wT": np.ascontiguousarray(W_lin.T).astype(bf16),
        "mT": mt_host.astype(bf16),
        "bf": np.tile(b_lin.reshape(1, H), (PT, 1)).astype(bf16),
    }
    # x: per-core, shifted tiles, [16, p(d%128), dc*t]; tile 0 = merged
    # edges: t L-56..L-1 at partitions 0..55, t 0..71 at partitions 56..127.
    xs = np.empty((B, TB, PT, DC * PT), np.float32)
    for b in range(B):
        xb = x[b]                                      # [L, D]
        for k in range(TB):
            if k == 0:
                sl = np.concatenate([xb[L - OFF:L], xb[0:HEAD]], axis=0)
            else:
                sl = xb[128 * k - OFF:128 * k + HEAD]  # [128 t, D]
            # [t, (dc p)] -> [p, dc, t]
            blk = sl.T.reshape(DC, PT, PT).transpose(1, 0, 2)
            xs[b, k] = blk.reshape(PT, DC * PT)
    arrays = {"xsT": xs.reshape(B * TB, PT, DC * PT).astype(bf16)}
    for name, arr in per_core.items():
        arrays[name] = np.concatenate([arr] * NCORES, axis=0)
    return arrays


def _run(arrays):
    fn, in_names, out_names, zero_outs, _ = _get_runner()
    global_zero = [np.concatenate([z] * NCORES, axis=0) for z in zero_outs]
    args = [arrays[n] for n in in_names] + global_zero
    outs = fn(*args)
    return {n: np.asarray(o) for n, o in zip(out_names, outs)}


def kernel(x, W_lin, b_lin, mem_w, la_w, gamma, beta):
    x = np.asarray(x, np.float32)
    W_lin = np.asarray(W_lin, np.float32)
    b_lin = np.asarray(b_lin, np.float32)
    wm = np.asarray(mem_w, np.float32).sum(axis=-1, dtype=np.float32)
    wf = np.asarray(la_w, np.float32).sum(axis=-1, dtype=np.float32)
    gamma = np.asarray(gamma, np.float32)
    beta = np.asarray(beta, np.float32)

    arrays = _prepare_in_arrays(x, W_lin, b_lin, wm, wf)
    outs = _run(arrays)
    out = outs["out"].reshape(NCORES, L, H)

    # gamma/beta affine (trivial for the spec's ones/zeros fills; exact in general)
    if not np.all(gamma == 1.0):
        out = out * gamma[None, None, :]
    if not np.all(beta == 0.0):
        out = out + beta[None, None, :]
    return np.ascontiguousarray(out.astype(np.float32))


# revision 19
# speedup vs baseline: 1.4264x; 1.4264x over previous
"""DFSMN layer Trainium2 kernel (8-core SPMD, batch-parallel).

Math: per batch b,
  h = x @ W^T + b_lin                      [L, H]
  out_pre[t] = h[t] + mem[t] + fut[t]  ==  (M @ h)[t]
    with M [L, L] banded: identity + past taps (50) + future taps (5),
    taps are scalars per lag: wm = mem_w.sum(-1), wf = la_w.sum(-1).
  out = LayerNorm_H(out_pre) * gamma + beta

On device (per core = one batch):
  Stage A produces g on a grid SHIFTED by -56: E_j holds g rows
  t in [128j-56, 128j+72); the edge halves (t in [1992,2048) and
  [0,72)) pack exactly into ONE merged tile gm (=tile 0, 56+72=128
  partitions, no padding). The Linear bias is folded into the stage-A
  PSUM evacuation as a DVE broadcast add (b shipped pre-tiled to
  [128, H]).

  Band: output tile j's UPPER 64 rows (t in [128j,128j+64)) need src
  t in [128j-50, 128j+68] which E_j covers; the LOWER 64 rows need
  src in [128j+14, 128j+132] covered by O_j = [128j+8, 128j+136),
  assembled from E_j[64:128] and E_{j+1}[0:64] by two partition-
  shifting SBUF->SBUF DMAs. So the band is TWO M=64 K=128 matmuls per
  (tile, h-chunk) issued to different PE column groups (out strips
  0:63 / 64:127) which stream CONCURRENTLY -- about one matmul-span
  of PE time instead of three matmuls + a rank-1 bias matmul.
  LayerNorm via DVE bn_stats/bn_aggr as before.
"""
import numpy as np
import ml_dtypes

MEM, LA, EPS = 50, 5, 1e-5
B, L, D, H = 8, 2048, 1024, 2048
NCORES = 8
PT = 128              # time tile (partition dim)
TB = L // PT          # 16 output time tiles
DC = D // PT          # 8 contract chunks
HN = 512              # matmul moving free dim
HC = H // HN          # 4 H chunks
OFF = 56              # source grid shift: E_j = [128j-OFF, 128j+128-OFF)
HEAD = PT - OFF       # 72: head rows of tile 0; tail rows = OFF = 56
NMT = 2 * TB          # band blocks: (ma_j, mb_j) per output tile

_cached = {}
last_exec_time_ns = None


def _band_matrix(wm, wf):
    """M [L, L] fp32: out_pre = M @ h."""
    M = np.zeros((L, L), np.float32)
    idx = np.arange(L)
    M[idx, idx] = 1.0
    for t in range(L):
        if t < MEM:
            M[t, :t] += wm[:t]
        else:
            M[t, t - MEM:t] += wm
        hi = min(t + LA, L - 1)
        if hi >= t + 1:
            M[t, t + 1:hi + 1] += wf[:hi - t]
    return M


def _src_t(k, p):
    """t index held at partition p of source tile k (merged gm at k=0:
    partitions 0..OFF-1 hold the tail t in [L-OFF, L), partitions
    OFF..127 hold the head t in [0, HEAD)). Returns None if out of
    range [0, L)."""
    if k == 0:
        t = (L - OFF + p) if p < OFF else (p - OFF)
    else:
        t = 128 * k - OFF + p
    return t if 0 <= t < L else None


def _build_nc(reps=1, loop_k=None):
    from concourse import bacc
    import concourse.mybir as mybir
    import concourse.tile as tile

    dt = mybir.dt.bfloat16
    f32 = mybir.dt.float32
    sub = mybir.AluOpType.subtract
    mult = mybir.AluOpType.mult
    add = mybir.AluOpType.add

    nc = bacc.Bacc(None, target_bir_lowering=False)
    # x shipped pre-transposed, shifted-tile-major, partition(d%128)-major:
    # xsT[k] is [128, DC*PT] with per-partition-contiguous 2KB lines.
    xsT = nc.declare_dram_parameter("xsT", [TB, PT, DC * PT], dt, isOutput=False)
    wT = nc.declare_dram_parameter("wT", [D, H], dt, isOutput=False)
    mT = nc.declare_dram_parameter("mT", [PT, NMT, 64], dt, isOutput=False)
    bf = nc.declare_dram_parameter("bf", [PT, H], dt, isOutput=False)
    out = nc.declare_dram_parameter("out", [L, H], f32, isOutput=True)

    with tile.TileContext(nc) as tc:
        with tc.tile_pool(name="const", bufs=1) as const, \
             tc.tile_pool(name="gpool", bufs=4) as gpool, \
             tc.tile_pool(name="owin", bufs=3) as owin, \
             tc.tile_pool(name="opool", bufs=3) as opool, \
             tc.tile_pool(name="ln", bufs=2) as ln, \
             tc.tile_pool(name="psg", bufs=4, space="PSUM") as psg, \
             tc.tile_pool(name="psp", bufs=3, space="PSUM") as psp:

            # Input DMAs round-robin over 4 engine queues, first-needed
            # first: tile 0's x slice + the hc=0 weight chunks gate the
            # first matmul chain (~1.25MB), everything else streams in
            # behind compute.
            qs = [nc.sync, nc.scalar, nc.gpsimd]
            qi = 0

            def q():
                nonlocal qi
                e = qs[qi % len(qs)]
                qi += 1
                return e

            xs_tiles = []
            for k in range(TB):
                t = const.tile([PT, DC, PT], dt, tag=f"xs{k}")
                xs_tiles.append(t)
            wt_tiles = {}
            for hc in range(HC):
                for dc in range(DC):
                    w = const.tile([PT, HN], dt, tag=f"wt{dc}_{hc}")
                    wt_tiles[(dc, hc)] = w

            q().dma_start(out=xs_tiles[0],
                          in_=xsT[0].rearrange("p (dc t) -> p dc t", dc=DC))
            for dc in range(DC):
                q().dma_start(out=wt_tiles[(dc, 0)],
                              in_=wT[dc * PT:(dc + 1) * PT, 0:HN])
            q().dma_start(out=xs_tiles[1],
                          in_=xsT[1].rearrange("p (dc t) -> p dc t", dc=DC))
            for hc in range(1, HC):
                for dc in range(DC):
                    q().dma_start(out=wt_tiles[(dc, hc)],
                                  in_=wT[dc * PT:(dc + 1) * PT,
                                         hc * HN:(hc + 1) * HN])
            for k in range(2, TB):
                q().dma_start(out=xs_tiles[k],
                              in_=xsT[k].rearrange("p (dc t) -> p dc t", dc=DC))
            mt_t = const.tile([PT, NMT, 64], dt, tag="mt")
            q().dma_start(out=mt_t, in_=mT[:, :, :])
            bf_t = const.tile([PT, H], dt, tag="bf")
            q().dma_start(out=bf_t, in_=bf[:, :])
            eps_t = const.tile([PT, 1], f32, tag="eps")
            nc.vector.memset(eps_t, EPS)

            consts = (xs_tiles, wt_tiles, mt_t, bf_t, eps_t)
            pools = (gpool, owin, opool, ln, psg, psp)
            ops = (sub, mult, add)
            if loop_k is not None:
                # For_i places an all-engine barrier at each iteration
                # boundary (PE sits idle through the LN/DMA tail, then
                # restarts cold). Unroll so the barrier is paid once per
                # UNROLL bodies; in between, bodies pipeline through the
                # tile rings.
                UNROLL = 8
                assert loop_k % UNROLL == 0
                with tc.For_i(0, loop_k // UNROLL, 1):
                    for _u in range(UNROLL):
                        _emit_body(nc, mybir, consts, pools, out, ops)
            else:
                for _rep in range(reps):
                    _emit_body(nc, mybir, consts, pools, out, ops)
    nc.finalize()
    return nc


def _emit_body(nc, mybir, consts, pools, out, ops):
    dt = mybir.dt.bfloat16
    f32 = mybir.dt.float32
    sub, mult, add = ops
    xs_tiles, wt_tiles, mt_t, bf_t, eps_t = consts
    gpool, owin, opool, ln, psg, psp = pools
    oq = [nc.sync, nc.scalar, nc.gpsimd]

    # g source tiles: k=0 (gm) lives in its own buffers for the whole
    # body (read by band j=0 AND the O_15 window); k=1..15 rotate.
    g_sb = [None] * TB

    def emit_A(k):
        gch = []
        for hc in range(HC):
            pg = psg.tile([PT, HN], f32, tag="pg")
            for dc in range(DC):
                nc.tensor.matmul(
                    pg,
                    xs_tiles[k][:, dc, :],
                    wt_tiles[(dc, hc)],
                    start=(dc == 0), stop=(dc == DC - 1))
            # gm (k=0) gets its own tags: it must survive until band j=15.
            tag = f"gm{hc}" if k == 0 else f"g{hc}"
            g = gpool.tile([PT, HN], dt, tag=tag)
            # Fold the Linear bias into the evacuation: g = psum + b.
            nc.vector.tensor_tensor(
                out=g, in0=pg, in1=bf_t[:, hc * HN:(hc + 1) * HN], op=add)
            gch.append(g)
        g_sb[k] = gch

    ow_tiles = [None] * TB

    def emit_OW(j):
        # O_j window: partitions 0:64 <- E_j[64:128], 64:128 <- E_{j+1}[0:64]
        # (j=15: 64:120 <- gm[0:56]; the K=120 matmul never reads 120:128).
        # Emitted a full super-iteration before B(j) consumes it so the
        # SBUF->SBUF DMA latency never gates the PE.
        kb = 64 + OFF if j == TB - 1 else PT   # valid lower-src partitions
        o_ch = []
        for hc in range(HC):
            ow = owin.tile([PT, HN], dt, tag=f"ow{hc}")
            oq[(j + hc) % 3].dma_start(out=ow[0:64, :],
                                       in_=g_sb[j][hc][64:128, :])
            src = g_sb[0][hc][0:OFF, :] if j == TB - 1 \
                else g_sb[j + 1][hc][0:64, :]
            oq[(j + hc + 1) % 3].dma_start(out=ow[64:kb, :], in_=src)
            o_ch.append(ow)
        ow_tiles[j] = o_ch

    def emit_B(j):
        kb = 64 + OFF if j == TB - 1 else PT
        o_ch = ow_tiles[j]
        stats = ln.tile([PT, HC, 6], f32, tag="stats")
        presb_ch = []
        for hc in range(HC):
            pre = psp.tile([PT, HN], f32, tag="pre")
            # Upper/lower 64-row halves on different PE column groups --
            # they stream concurrently (out strips 0:63 and 64:127).
            nc.tensor.matmul(pre[0:64, :], mt_t[:, 2 * j, :], g_sb[j][hc],
                             start=True, stop=True)
            nc.tensor.matmul(pre[64:128, :], mt_t[0:kb, 2 * j + 1, :],
                             o_ch[hc][0:kb, :], start=True, stop=True,
                             skip_group_check=True)
            # Evacuate PSUM on ScalarE (close to PSUM); LN from SBUF.
            pre_sb = opool.tile([PT, HN], f32, tag=f"presb{hc}")
            nc.scalar.copy(out=pre_sb, in_=pre)
            nc.vector.bn_stats(out=stats[:, hc, :], in_=pre_sb)
            presb_ch.append(pre_sb)
        mv = ln.tile([PT, 2], f32, tag="mv")
        nc.vector.bn_aggr(out=mv, in_=stats)
        rstd = ln.tile([PT, 1], f32, tag="rstd")
        nc.scalar.activation(
            out=rstd, in_=mv[:, 1:2],
            func=mybir.ActivationFunctionType.Sqrt,
            bias=eps_t, scale=1.0)
        nc.vector.reciprocal(out=rstd, in_=rstd)
        # Per-chunk LN apply + chunked out-DMA on rotating queues: the
        # DMA of chunk hc overlaps the tensor_scalar of chunk hc+1, and
        # the iteration tail is one 256KB DMA instead of a 1MB one.
        o = opool.tile([PT, HC, HN], f32, tag="o")
        for hc in range(HC):
            nc.vector.tensor_scalar(
                out=o[:, hc, :], in0=presb_ch[hc],
                scalar1=mv[:, 0:1], scalar2=rstd,
                op0=sub, op1=mult)
            oq[(4 * j + hc) % 3].dma_start(
                out=out[j * PT:(j + 1) * PT, hc * HN:(hc + 1) * HN],
                in_=o[:, hc, :])

    # A(k); O_{k-1} windows (DMA, one super-iteration of slack); B(k-2).
    emit_A(0)
    for k in range(1, TB):
        emit_A(k)
        emit_OW(k - 1)
        if k >= 2:
            emit_B(k - 2)
    emit_OW(TB - 1)
    emit_B(TB - 2)
    emit_B(TB - 1)


def _get_runner(reps=1):
    """Compile once; return (run_fn, in_names, out_names).

    run_fn takes a list of global (concatenated-over-cores) jax/np arrays in
    in_names order followed by zero output buffers, returns global outputs.
    Mirrors concourse.bass2jax.run_bass_via_pjrt's multi-core branch, but
    keeps the jitted callable so repeated invocations don't rebuild/retrace.
    """
    key = ("runner", reps)
    if key in _cached:
        return _cached[key]

    import jax
    from jax.experimental.shard_map import shard_map
    from jax.sharding import Mesh, PartitionSpec
    import concourse.mybir as mybir
    from concourse import bass2jax

    if isinstance(reps, tuple):  # ("loop", K): hardware For_i timing variant
        nc = _build_nc(loop_k=reps[1])
    else:
        nc = _build_nc(reps)
    bass2jax.install_neuronx_cc_hook()

    partition_name = nc.partition_id_tensor.name if nc.partition_id_tensor else None
    in_names, out_names, out_avals, zero_outs = [], [], [], []
    for alloc in nc.m.functions[0].allocations:
        if not isinstance(alloc, mybir.MemoryLocationSet):
            continue
        name = alloc.memorylocations[0].name
        if alloc.kind == "ExternalInput":
            if name != partition_name:
                in_names.append(name)
        elif alloc.kind == "ExternalOutput":
            out_names.append(name)
            shape = tuple(alloc.tensor_shape)
            dtype = mybir.dt.np(alloc.dtype)
            out_avals.append(jax.core.ShapedArray(shape, dtype))
            zero_outs.append(np.zeros(shape, dtype))
    n_params = len(in_names)
    all_names = in_names + out_names
    if partition_name is not None:
        all_names.append(partition_name)

    def _body(*args):
        operands = list(args)
        if partition_name is not None:
            operands.append(bass2jax.partition_id_tensor())
        outs = bass2jax._bass_exec_p.bind(
            *operands,
            out_avals=tuple(out_avals),
            in_names=tuple(all_names),
            out_names=tuple(out_names),
            lowering_input_output_aliases=(),
            sim_require_finite=True,
            sim_require_nnan=True,
            nc=nc,
        )
        return tuple(outs)

    devices = jax.devices()[:NCORES]
    assert len(devices) == NCORES, f"need {NCORES} devices, have {len(jax.devices())}"
    mesh = Mesh(np.asarray(devices), ("core",))
    n_outs = len(out_names)
    fn = jax.jit(shard_map(
        _body, mesh=mesh,
        in_specs=(PartitionSpec("core"),) * (n_params + n_outs),
        out_specs=(PartitionSpec("core"),) * n_outs,
        check_rep=False))

    _cached[key] = (fn, in_names, out_names, zero_outs, mesh)
    return _cached[key]


def _prepare_in_arrays(x, W_lin, b_lin, wm, wf):
    """Host prep: per-core inputs concatenated over the core axis (axis 0)."""
    bf16 = ml_dtypes.bfloat16
    M = _band_matrix(wm, wf)
    # ma_j[p, q] = M[128j+q,       src_t(j, p)]   (upper 64 out rows, E_j)
    # mb_j[p, q] = M[128j+64+q, 128j+8+p]         (lower 64 out rows, O_j)
    mt_host = np.zeros((PT, NMT, 64), np.float32)
    for j in range(TB):
        for p in range(PT):
            t = _src_t(j, p)
            if t is not None:
                mt_host[p, 2 * j, :] = M[j * PT:j * PT + 64, t]
        lo = 128 * j + 8
        n = min(PT, L - lo)                        # 120 for j=15
        mt_host[0:n, 2 * j + 1, :] = M[j * PT + 64:(j + 1) * PT, lo:lo + n].T
    per_core = {
        "wT": np.ascontiguousarray(W_lin.T).astype(bf16),
        "mT": mt_host.astype(bf16),
        "bf": np.tile(b_lin.reshape(1, H), (PT, 1)).astype(bf16),
    }
    # x: per-core, shifted tiles, [16, p(d%128), dc*t]; tile 0 = merged
    # edges: t L-56..L-1 at partitions 0..55, t 0..71 at partitions 56..127.
    xs = np.empty((B, TB, PT, DC * PT), np.float32)
    for b in range(B):
        xb = x[b]                                      # [L, D]
        for k in range(TB):
            if k == 0:
                sl = np.concatenate([xb[L - OFF:L], xb[0:HEAD]], axis=0)
            else:
                sl = xb[128 * k - OFF:128 * k + HEAD]  # [128 t, D]
            # [t, (dc p)] -> [p, dc, t]
            blk = sl.T.reshape(DC, PT, PT).transpose(1, 0, 2)
            xs[b, k] = blk.reshape(PT, DC * PT)
    arrays = {"xsT": xs.reshape(B * TB, PT, DC * PT).astype(bf16)}
    for name, arr in per_core.items():
        arrays[name] = np.concatenate([arr] * NCORES, axis=0)
    return arrays


def _run(arrays):
    fn, in_names, out_names, zero_outs, _ = _get_runner()
    global_zero = [np.concatenate([z] * NCORES, axis=0) for z in zero_outs]
    args = [arrays[n] for n in in_names] + global_zero
    outs = fn(*args)
    return {n: np.asarray(o) for n, o in zip(out_names, outs)}


def kernel(x, W_lin, b_lin, mem_w, la_w, gamma, beta):
    x = np.asarray(x, np.float32)
    W_lin = np.asarray(W_lin, np.float32)
    b_lin = np.asarray(b_lin, np.float32)
    wm = np.asarray(mem_w, np.float32).sum(axis=-1, dtype=np.float32)
    wf = np.asarray(la_w, np.float32).sum(axis=-1, dtype=np.float32)
    gamma = np.asarray(gamma, np.float32)
    beta = np.asarray(beta, np.float32)

    arrays = _prepare_in_arrays(x, W_lin, b_lin, wm, wf)
    outs = _run(arrays)
    out = outs["out"].reshape(NCORES, L, H)

    # gamma/beta affine (trivial for the spec's ones/zeros fills; exact in general)
    if not np.all(gamma == 1.0):
        out = out * gamma[None, None, :]
    if not np.all(beta == 0.0):
        out = out + beta[None, None, :]
    return np.ascontiguousarray(out.astype(np.float32))
